# revision 14
# baseline (speedup 1.0000x reference)
"""Cross-activation regularization loss kernel for Trainium2 (8 NeuronCores).

Math (per sample b of x[B=64, T=4096, F=128]):
    G  = x_b^T x_b                (Gram over time, [F,F], PE matmuls in bf16)
    s' = x_b^T (1/sqrt(T))        (scaled column sums, [F,1])
    C  = G - s' s'^T              (the 1/T factors cancel in the correlation)
    r  = 1/sqrt(diag(C))
    corr_b = diag(r) C diag(r)    (two matmuls against D_r = I*r)
Each core handles 8 samples (data-parallel over batch), accumulates
corr_sum = sum_b corr_b and per-sample L1 partials; the host sums partials
across cores and computes the 3 output scalars.
"""

from contextlib import ExitStack

import numpy as np

import concourse.bacc as bacc
import concourse.bass as bass
import concourse.bass_utils as bass_utils
import concourse.tile as tile
from concourse import mybir
from concourse.bass import ts
from concourse.masks import make_identity

N_CORES = 8
B = 64
B_PER_CORE = B // N_CORES  # 8
T = 4096
F = 128
NCHUNK = T // 128  # 32
CROSS_ACTIVATION_LAMBDA = 0.01

_f32 = mybir.dt.float32
_bf16 = mybir.dt.bfloat16


def _build(loop_m=None, bufs=2, dma_split=1, dma_cast=False):
    nc = bacc.Bacc("TRN2", target_bir_lowering=False, debug=False,
                   num_devices=N_CORES)
    x_d = nc.dram_tensor("x_local", [B_PER_CORE, T, F], _f32,
                         kind="ExternalInput")
    corr_d = nc.dram_tensor("corr_sum", [F, F], _f32, kind="ExternalOutput")
    l1_d = nc.dram_tensor("l1part", [F, B_PER_CORE * dma_split], _f32,
                          kind="ExternalOutput")

    # t = 32*p + n: partition p holds 32 consecutive time rows (contiguous DMA);
    # chunk n contracts t over partitions, summing chunks covers all t.
    xr = x_d.ap().rearrange("b (p n) f -> b p (n f)", p=128)

    with tile.TileContext(nc) as tc, ExitStack() as ctx:
        singles = ctx.enter_context(tc.tile_pool(name="singles", bufs=1))
        xpool = ctx.enter_context(tc.tile_pool(name="x", bufs=bufs))
        bfpool = ctx.enter_context(tc.tile_pool(name="bf", bufs=bufs))
        scrpool = ctx.enter_context(tc.tile_pool(name="scr", bufs=bufs))
        work = ctx.enter_context(tc.tile_pool(name="work", bufs=2))
        psG = ctx.enter_context(tc.tile_pool(name="psG", bufs=2, space="PSUM"))
        psS = ctx.enter_context(tc.tile_pool(name="psS", bufs=2, space="PSUM"))
        psE = ctx.enter_context(tc.tile_pool(name="psE", bufs=1, space="PSUM"))

        ident = singles.tile([F, F], _f32)
        make_identity(nc, ident)
        cv = singles.tile([128, 1], _bf16)
        nc.vector.memset(cv, 1.0 / 64.0)  # 1/sqrt(T), exactly representable
        corr_sb = singles.tile([F, F], _f32)
        nc.vector.memset(corr_sb, 0.0)
        l1_sb = singles.tile([F, B_PER_CORE * dma_split], _f32)

        loop_cm = tc.For_i(0, loop_m, 1) if loop_m is not None else None
        if loop_cm is not None:
            loop_cm.__enter__()

        for b in range(B_PER_CORE):
            xt = (None if dma_cast else
                  xpool.tile([128, NCHUNK * F], _f32, tag="xt"))
            xb = bfpool.tile([128, NCHUNK * F], _bf16, tag="xb")
            abs_scr = scrpool.tile([128, NCHUNK * F], _bf16, tag="abs_scr")
            piece = NCHUNK * F // dma_split
            for k in range(dma_split):
                sl = slice(k * piece, (k + 1) * piece)
                l1slot = l1_sb[:, b * dma_split + k: b * dma_split + k + 1]
                if dma_cast:
                    # SWDGE casts fp32->bf16 in the DMA datapath; L1 from bf16
                    nc.gpsimd.dma_start(out=xb[:, sl], in_=xr[b][:, sl])
                    nc.scalar.activation(
                        out=abs_scr[:, sl], in_=xb[:, sl],
                        func=mybir.ActivationFunctionType.Abs,
                        accum_out=l1slot)
                else:
                    nc.sync.dma_start(out=xt[:, sl], in_=xr[b][:, sl])
                    # L1 partial on ACT: |x| + free-dim accumulate
                    nc.scalar.activation(
                        out=abs_scr[:, sl], in_=xt[:, sl],
                        func=mybir.ActivationFunctionType.Abs,
                        accum_out=l1slot)
                    # bf16 cast for the PE matmuls
                    nc.vector.tensor_copy(out=xb[:, sl], in_=xt[:, sl])

            gs = psG.tile([F, F], _f32, tag="G")
            sp = psS.tile([F, 1], _f32, tag="s")
            for n in range(NCHUNK):
                chunk = xb[:, ts(n, F)]
                nc.tensor.matmul(gs, chunk, chunk,
                                 start=(n == 0), stop=(n == NCHUNK - 1))
                nc.tensor.matmul(sp, chunk, cv,
                                 start=(n == 0), stop=(n == NCHUNK - 1))

            # ---- per-sample epilogue (fp32) ----
            s_sb = work.tile([F, 1], _f32, tag="s_sb")
            nc.scalar.copy(out=s_sb, in_=sp)
            sT_ps = psE.tile([1, F], _f32, tag="sT")
            nc.tensor.matmul(sT_ps, s_sb, ident)      # s'^T as a row
            sT_sb = work.tile([1, F], _f32, tag="sT_sb")
            nc.vector.tensor_copy(out=sT_sb, in_=sT_ps)
            outer_ps = psE.tile([F, F], _f32, tag="outer")
            nc.tensor.matmul(outer_ps, sT_sb, sT_sb)  # s' s'^T
            C_sb = work.tile([F, F], _f32, tag="C")
            nc.vector.tensor_copy(out=C_sb, in_=gs)
            nc.vector.tensor_sub(out=C_sb, in0=C_sb, in1=outer_ps)

            dtmp = work.tile([F, F], _f32, tag="dtmp")
            dvec = work.tile([F, 1], _f32, tag="dvec")
            nc.vector.tensor_mul(out=dtmp, in0=C_sb, in1=ident)
            nc.vector.reduce_sum(out=dvec, in_=dtmp,
                                 axis=mybir.AxisListType.X)
            rec = work.tile([F, 1], _f32, tag="rec")
            nc.vector.reciprocal(out=rec, in_=dvec)
            r = work.tile([F, 1], _f32, tag="r")
            nc.scalar.sqrt(out=r, in_=rec)            # r = 1/sqrt(d)
            dr = work.tile([F, F], _f32, tag="dr")
            nc.vector.tensor_scalar_mul(out=dr, in0=ident, scalar1=r)

            p1_ps = psE.tile([F, F], _f32, tag="p1")
            nc.tensor.matmul(p1_ps, C_sb, dr)         # C D
            p1_sb = work.tile([F, F], _f32, tag="p1_sb")
            nc.vector.tensor_copy(out=p1_sb, in_=p1_ps)
            p2_ps = psE.tile([F, F], _f32, tag="p2")
            nc.tensor.matmul(p2_ps, p1_sb, dr)        # D C D = corr_b
            nc.vector.tensor_add(out=corr_sb, in0=corr_sb, in1=p2_ps)

        if loop_cm is not None:
            loop_cm.__exit__(None, None, None)

        nc.sync.dma_start(out=corr_d.ap(), in_=corr_sb)
        nc.sync.dma_start(out=l1_d.ap(), in_=l1_sb)

    nc.compile()
    return nc


_nc_cache = None


def _get_nc():
    global _nc_cache
    if _nc_cache is None:
        _nc_cache = _build(bufs=2, dma_split=2, dma_cast=True)
    return _nc_cache


def _run(x, **spmd_kwargs):
    x = np.ascontiguousarray(np.asarray(x, dtype=np.float32))
    assert x.shape == (B, T, F), x.shape
    nc = _get_nc()
    in_maps = [{"x_local": x[c * B_PER_CORE:(c + 1) * B_PER_CORE]}
               for c in range(N_CORES)]
    return bass_utils.run_bass_kernel_spmd(
        nc, in_maps, core_ids=list(range(N_CORES)), **spmd_kwargs)


def _finalize(results):
    corr = np.zeros((F, F), dtype=np.float64)
    l1 = 0.0
    for res in results:
        corr += res["corr_sum"].astype(np.float64)
        l1 += float(res["l1part"].astype(np.float64).sum())
    avg_abs = np.abs(corr / B)
    tri_sum = float(np.triu(avg_abs, k=1).sum())
    n_pairs = F * (F - 1) // 2
    loss = tri_sum * CROSS_ACTIVATION_LAMBDA / n_pairs
    activity_l1 = l1 / F
    return np.array([loss, tri_sum, activity_l1], dtype=np.float32)


def kernel(inputs):
    br = _run(inputs)
    return _finalize(br.results)
